# revision 11
# baseline (speedup 1.0000x reference)
"""Trainium2 Bass kernel for DirectionalHMAGAT message passing.

Contract: kernel(**inputs) takes full unsharded numpy inputs, returns the
full [N, H*C] float32 output. Internally shards edges across 8 NeuronCores
by destination-node range and runs one SPMD Bass program.

Design (v2):
- Edges sorted by dst, grouped into 49 windows of exactly 128 dst nodes per
  core. Each window g has S[g] sub-batches of 128 edge slots (S[g] = max
  over cores, so the SPMD program is shape-uniform).
- Per sub-batch, one SWDGE indirect gather pulls the 128 source rows of x
  (bf16, 128 B/row). The Pool engine's descriptor generation (~1.1 us per
  gather) is the pipeline wall; everything else overlaps under it.
- Attention scores: v = xwinT @ (W_att/SCALE) once per window; per-edge
  keys vd = onehotT @ v via PE; score = reduce(vd * xsrc). Softmax runs
  without max-subtraction (mathematically identical, scores are O(1)).
- Aggregation: numer[j, :] = sum_e onehot[e, j] * (xsrc*ew | ew) via PE
  accumulation in PSUM; the H*F->H*C projection + 1/denom + bias are fused
  in-group (no DRAM round trip, no scatters: windows are contiguous rows).
"""

import json

import ml_dtypes
import numpy as np

from concourse import bass, mybir
from concourse.bass import IndirectOffsetOnAxis
from concourse.bass_utils import run_bass_kernel_spmd
from concourse.masks import make_identity
from concourse.tile import TileContext


def _legalize_sync_waits(bir: bytes) -> bytes:
    """The walrus build in this image accepts at most one sync wait per
    instruction; Tile emits several. Hoist the extras onto single-wait NoOps
    inserted just before the instruction on the same engine."""
    m = json.loads(bir)
    k = 0
    changed = False
    for fn in m["functions"]:
        for b in fn["blocks"]:
            out = []
            for inst in b["instructions"]:
                sy = inst.get("sync_info")
                waits = sy.get("on_wait") if sy else None
                if waits and len(waits) > 1:
                    changed = True
                    for w in waits[:-1]:
                        k += 1
                        out.append({
                            "debug": inst.get("debug"),
                            "engine": inst["engine"],
                            "ins": [],
                            "outs": [],
                            "name": f"I-waitfix-{k}",
                            "opcode": "NoOp",
                            "sync_info": {"on_update": [], "on_wait": [w]},
                        })
                    sy["on_wait"] = [waits[-1]]
                out.append(inst)
            b["instructions"] = out
    if not changed:
        return bir
    return json.dumps(m).encode()


if not getattr(bass.Bass, "_waitfix_patched", False):
    _orig_to_json_bytes = bass.Bass.to_json_bytes

    def _to_json_bytes_fixed(self):
        return _legalize_sync_waits(_orig_to_json_bytes(self))

    bass.Bass.to_json_bytes = _to_json_bytes_fixed
    bass.Bass._waitfix_patched = True

# Problem constants (hardcoded per harness contract)
N, F, H, C, E = 50000, 64, 4, 64, 800000
SCALE = float(np.sqrt(F))
NEG = 0.2
NCORES = 8
NPC = 6272            # nodes per core = 49 * 128 (8 * 6272 = 50176 >= N)
NWIN = NPC // 128     # 49 windows of 128 dst nodes
SMAX = 22             # hard cap on sub-batches per window (tile allocation)
NPAD = NCORES * NPC   # 50176
HF = H * F            # 256
NUMW = HF + H         # 260: numerator (256) + denominator (4)

f32 = mybir.dt.float32
bf16 = mybir.dt.bfloat16
i32 = mybir.dt.int32
i8 = mybir.dt.int8


def _prep_edges(edge_index, edge_weight):
    """Sort edges by dst, shard by dst range, pack windows of 128 dst nodes.

    Returns (S, itile, dstloc, wt, dstrow) where S[g] is the common
    sub-batch count of window g and the arrays are per-core metadata.
    """
    src = np.ascontiguousarray(edge_index[0]).astype(np.int64)
    dst = np.ascontiguousarray(edge_index[1]).astype(np.int64)
    w = np.ascontiguousarray(edge_weight[:, 0]).astype(np.float32)

    per_core = []
    counts = np.zeros((NCORES, NWIN), np.int64)
    for c in range(NCORES):
        lo, hi = c * NPC, (c + 1) * NPC
        m = (dst >= lo) & (dst < hi)
        s_c, d_c, w_c = src[m], dst[m] - lo, w[m]
        o = np.argsort(d_c, kind="stable")
        s_c, d_c, w_c = s_c[o], d_c[o], w_c[o]
        bounds = np.searchsorted(d_c, np.arange(NWIN + 1) * 128)
        counts[c] = bounds[1:] - bounds[:-1]
        per_core.append((s_c, d_c, w_c, bounds))

    S = np.maximum(1, -(-counts.max(axis=0) // 128))  # per-window sub-batches
    if S.max() > SMAX:
        raise ValueError(f"window needs {S.max()} sub-batches > SMAX={SMAX}")
    OFS = np.concatenate([[0], np.cumsum(S)])
    TOT = int(OFS[-1])

    itile = np.zeros((NCORES, 128, TOT), np.int32)
    wt = np.zeros((NCORES, 128, TOT), ml_dtypes.bfloat16)
    dstrow = np.zeros((NCORES, 128, TOT * 128), np.int8)
    dstcol = np.zeros((NCORES, 128, TOT * 128), np.int8)
    for c in range(NCORES):
        s_c, d_c, w_c, bounds = per_core[c]
        for g in range(NWIN):
            st, en = bounds[g], bounds[g + 1]
            n = en - st
            if n == 0:
                continue
            k = np.arange(n)
            p, b = k % 128, k // 128
            col = OFS[g] + b
            itile[c, p, col] = s_c[st:en]
            dl = (d_c[st:en] - g * 128).astype(np.float32)
            wt[c, p, col] = w_c[st:en].astype(ml_dtypes.bfloat16)
            # ohT[j, b*128+q] = (dl(q,b) == j): value replicated down partitions
            dr = np.zeros(int(S[g]) * 128, np.float32)
            dr[b * 128 + p] = dl
            # pad slots keep dstloc 0 / w 0: their ew is 0 so they only add
            # exact zeros into window row 0
            dstrow[c, :, OFS[g] * 128:(OFS[g] + S[g]) * 128] = \
                dr[None, :].astype(np.int8)
            # oh[q, s*128+j] = (dl(q,s) == j): value replicated along j
            dc2 = np.zeros((128, int(S[g]), 128), np.float32)
            dc2[p, b, :] = dl[:, None]
            dstcol[c, :, OFS[g] * 128:(OFS[g] + S[g]) * 128] = \
                dc2.reshape(128, -1).astype(np.int8)
    return S, OFS, TOT, itile, dstcol, wt, dstrow


_build_cache = {}


def _build(S, OFS, TOT):
    key = tuple(int(s) for s in S)
    if key in _build_cache:
        return _build_cache[key]
    nc = bass.Bass(num_swdge_queues=4)
    xbf_d = nc.declare_dram_parameter("xbf", [NPAD, F], bf16, isOutput=False)
    watt_d = nc.declare_dram_parameter("watt", [F, HF], bf16, isOutput=False)
    wbd_d = nc.declare_dram_parameter("wbd", [2, 128, HF], bf16, isOutput=False)
    biasb_d = nc.declare_dram_parameter("biasb", [128, HF], f32, isOutput=False)
    itile_d = nc.declare_dram_parameter("itile", [128, TOT], i32, isOutput=False)
    dstcol_d = nc.declare_dram_parameter("dstcol", [128, TOT * 128], i8, isOutput=False)
    wt_d = nc.declare_dram_parameter("wt", [128, TOT], bf16, isOutput=False)
    dstrow_d = nc.declare_dram_parameter("dstrow", [128, TOT * 128], i8, isOutput=False)
    xwinT_d = nc.declare_dram_parameter("xwinT", [NWIN, F, 128], bf16, isOutput=False)
    out_d = nc.declare_dram_parameter("out", [NPC, HF], f32, isOutput=True)

    AT = mybir.ActivationFunctionType
    OP = mybir.AluOpType
    QN = ["qPoolDynamic", "qPoolDynamic1", "qPoolDynamic2", "qPoolDynamic3"]

    with TileContext(nc) as tc:
        with tc.tile_pool(name="const", bufs=1) as cp:
            watt_s = cp.tile([F, HF], bf16)
            nc.sync.dma_start(watt_s[:], watt_d[:])
            wbd_a = cp.tile([128, HF], bf16)
            nc.sync.dma_start(wbd_a[:], wbd_d[0])
            wbd_b = cp.tile([128, HF], bf16)
            nc.sync.dma_start(wbd_b[:], wbd_d[1])
            biasb = cp.tile([128, HF], f32)
            nc.sync.dma_start(biasb[:], biasb_d[:])
            identb = cp.tile([128, 128], bf16)
            make_identity(nc, identb[:])
            iota_i = cp.tile([128, SMAX, 128], i32)
            nc.gpsimd.iota(iota_i[:], pattern=[[0, SMAX], [1, 128]], base=0,
                           channel_multiplier=0)
            iotaf_rep = cp.tile([128, SMAX * 128], i8)
            nc.vector.tensor_copy(iotaf_rep[:], iota_i[:].rearrange("p s f -> p (s f)"))
            iotap_i = cp.tile([128, SMAX * 128], i32)
            nc.gpsimd.iota(iotap_i[:], pattern=[[0, SMAX * 128]], base=0,
                           channel_multiplier=1)
            iotap_rep = cp.tile([128, SMAX * 128], i8)
            nc.vector.tensor_copy(iotap_rep[:], iotap_i[:])

            with (
                tc.tile_pool(name="itp", bufs=10) as ip,
                tc.tile_pool(name="meta", bufs=3) as mp,
                tc.tile_pool(name="gx", bufs=6) as gp,
                tc.tile_pool(name="wk", bufs=2) as wp,
                tc.tile_pool(name="onehot", bufs=3) as op_,
                tc.tile_pool(name="small", bufs=3) as sp,
                tc.tile_pool(name="ps_vd", bufs=2, space="PSUM") as pv,
                tc.tile_pool(name="ps_v1", bufs=1, space="PSUM") as pv1,
                tc.tile_pool(name="ps_n", bufs=2, space="PSUM") as pn,
                tc.tile_pool(name="ps_t", bufs=1, space="PSUM") as pt,
            ):
                for g in range(NWIN):
                    Sg = int(S[g])
                    o0 = int(OFS[g])
                    # ---- meta loads ----
                    it = ip.tile([128, SMAX], i32, tag="it")
                    nc.sync.dma_start(it[:, :Sg], itile_d[:, o0:o0 + Sg])
                    dcol = mp.tile([128, SMAX * 128], i8, tag="dcol")
                    nc.sync.dma_start(dcol[:, :Sg * 128],
                                      dstcol_d[:, o0 * 128:(o0 + Sg) * 128])
                    wtt = mp.tile([128, SMAX], bf16, tag="wt")
                    nc.sync.dma_start(wtt[:, :Sg], wt_d[:, o0:o0 + Sg])
                    drow = mp.tile([128, SMAX * 128], i8, tag="drow")
                    nc.sync.dma_start(drow[:, :Sg * 128],
                                      dstrow_d[:, o0 * 128:(o0 + Sg) * 128])
                    xwT = mp.tile([F, 128], bf16, tag="xwT")
                    nc.sync.dma_start(xwT[:], xwinT_d[g])

                    # ---- gathers (the pipeline wall: ~1.1us each on Pool) ----
                    xsrc = gp.tile([128, SMAX, F], bf16, tag="xsrc")
                    for b in range(Sg):
                        gi = nc.gpsimd.indirect_dma_start(
                            out=xsrc[:, b, :], out_offset=None, in_=xbf_d[:],
                            in_offset=IndirectOffsetOnAxis(
                                ap=it[:, b:b + 1], axis=0),
                        )
                        gi.ins.queue = QN[b % 4]

                    # ---- per-window attention keys: v = xwinT^T @ watt ----
                    v_ps = pv1.tile([128, HF], f32, tag="v_ps")
                    nc.tensor.matmul(v_ps[:], lhsT=xwT[:], rhs=watt_s[:],
                                     start=True, stop=True)
                    v_bf = sp.tile([128, HF], bf16, tag="v_bf")
                    nc.scalar.copy(v_bf[:], v_ps[:])

                    # ---- one-hots ----
                    oh = op_.tile([128, SMAX, 128], bf16, tag="oh")
                    nc.vector.tensor_tensor(
                        oh[:, :Sg, :].rearrange("p s f -> p (s f)"),
                        dcol[:, :Sg * 128],
                        iotaf_rep[:, :Sg * 128],
                        op=OP.is_equal)
                    ohT = op_.tile([128, SMAX * 128], bf16, tag="ohT")
                    nc.vector.tensor_tensor(
                        ohT[:, :Sg * 128],
                        drow[:, :Sg * 128],
                        iotap_rep[:, :Sg * 128],
                        op=OP.is_equal)

                    # ---- vd = onehotT @ v  (per-edge keys), then score ----
                    vd_bf = wp.tile([128, SMAX, HF], bf16, tag="vd_bf")
                    for b in range(Sg):
                        vd_ps = pv.tile([128, HF], f32, tag="vd_ps")
                        nc.tensor.matmul(
                            vd_ps[:], lhsT=ohT[:, b * 128:(b + 1) * 128],
                            rhs=v_bf[:], start=True, stop=True)
                        nc.scalar.copy(vd_bf[:, b, :], vd_ps[:])
                    scr = wp.tile([128, SMAX, H, F], bf16, tag="scr")
                    nc.vector.tensor_tensor(
                        scr[:, :Sg, :, :],
                        vd_bf[:, :Sg, :].rearrange("p s (h f) -> p s h f", h=H),
                        xsrc[:, :Sg, :].rearrange("p s (o f) -> p s o f", o=1)
                        .to_broadcast([128, Sg, H, F]),
                        op=OP.mult)
                    score = sp.tile([128, SMAX * H], f32, tag="score")
                    nc.vector.tensor_reduce(
                        score[:, :Sg * H].rearrange("p (s h) -> p s h", h=H),
                        scr[:, :Sg, :, :].rearrange("p s h f -> p (s h) f"),
                        axis=mybir.AxisListType.X, op=OP.add)

                    # ---- softmax numerator weights ----
                    s02 = sp.tile([128, SMAX * H], f32, tag="s02")
                    nc.vector.tensor_scalar_mul(s02[:, :Sg * H], score[:, :Sg * H], NEG)
                    slr = sp.tile([128, SMAX * H], f32, tag="slr")
                    nc.vector.tensor_tensor(slr[:, :Sg * H], score[:, :Sg * H],
                                            s02[:, :Sg * H], op=OP.max)
                    e1 = sp.tile([128, SMAX * H], bf16, tag="e1")
                    nc.scalar.activation(e1[:, :Sg * H], slr[:, :Sg * H], AT.Exp)
                    ew = sp.tile([128, SMAX, H], bf16, tag="ew")
                    nc.vector.tensor_tensor(
                        ew[:, :Sg, :],
                        e1[:, :Sg * H].rearrange("p (s h) -> p s h", h=H),
                        wtt[:, :Sg].rearrange("p (s o) -> p s o", o=1)
                        .to_broadcast([128, Sg, H]),
                        op=OP.mult)

                    # ---- messages ----
                    rhs = wp.tile([128, SMAX, NUMW], bf16, tag="rhs")
                    nc.vector.tensor_tensor(
                        rhs[:, :Sg, 0:HF].rearrange("p s (h f) -> p s h f", h=H),
                        xsrc[:, :Sg, :].rearrange("p s (o f) -> p s o f", o=1)
                        .to_broadcast([128, Sg, H, F]),
                        ew[:, :Sg, :].rearrange("p s (h o) -> p s h o", o=1)
                        .to_broadcast([128, Sg, H, F]),
                        op=OP.mult)
                    nc.vector.tensor_copy(rhs[:, :Sg, HF:NUMW], ew[:, :Sg, :])

                    # ---- aggregate at dst (PSUM accumulate) ----
                    numer_ps = pn.tile([128, NUMW], f32, tag="numer")
                    for b in range(Sg):
                        nc.tensor.matmul(numer_ps[:], lhsT=oh[:, b, :],
                                         rhs=rhs[:, b, :],
                                         start=(b == 0), stop=(b == Sg - 1))

                    # ---- fused node pass ----
                    dn = sp.tile([128, H], f32, tag="dn")
                    nc.vector.tensor_scalar_add(dn[:], numer_ps[:, HF:NUMW], 1e-16)
                    rcp = sp.tile([128, H], f32, tag="rcp")
                    nc.vector.reciprocal(rcp[:], dn[:])
                    numer_bf = sp.tile([128, HF], bf16, tag="numer_bf")
                    nc.scalar.copy(numer_bf[:], numer_ps[:, 0:HF])
                    nt_ps = pt.tile([128, 2, 128], bf16, tag="nt_ps")
                    nc.tensor.transpose(nt_ps[:, 0, :], numer_bf[:, 0:128], identb[:])
                    nc.tensor.transpose(nt_ps[:, 1, :], numer_bf[:, 128:256], identb[:])
                    na = sp.tile([128, 2, 128], bf16, tag="na")
                    nc.scalar.copy(na[:], nt_ps[:])
                    out_ps = pt.tile([128, HF], f32, tag="out_ps")
                    nc.tensor.matmul(out_ps[:], lhsT=na[:, 0, :], rhs=wbd_a[:],
                                     start=True, stop=False)
                    nc.tensor.matmul(out_ps[:], lhsT=na[:, 1, :], rhs=wbd_b[:],
                                     start=False, stop=True)
                    outn = sp.tile([128, HF], f32, tag="outn")
                    nc.vector.tensor_tensor(
                        outn[:].rearrange("p (h c) -> p h c", h=H),
                        out_ps[:].rearrange("p (h c) -> p h c", h=H),
                        rcp[:].rearrange("p (h o) -> p h o", o=1)
                        .to_broadcast([128, H, C]),
                        op=OP.mult)
                    outt = sp.tile([128, HF], f32, tag="outt")
                    nc.vector.tensor_tensor(outt[:], outn[:], biasb[:], op=OP.add)
                    nc.scalar.dma_start(out_d[g * 128:(g + 1) * 128, :], outt[:])
    _build_cache[key] = nc
    return nc


def _make_in_maps(x, W_lin, W_att, bias, itile, dstcol, wt, dstrow):
    xbf = np.zeros((NPAD, F), ml_dtypes.bfloat16)
    xbf[:N] = np.asarray(x, np.float32).astype(ml_dtypes.bfloat16)
    # v[j, h, f'] must contract W_att's FIRST index with src features:
    # score = sum_{f',f} x[src,f'] W_att[f',hF+f] x[dst,f]
    #       = sum_{f'} x[src,f'] * v[dst,h,f'] with v = x_dst @ W_att^T(per head)
    wa = (np.asarray(W_att, np.float32) / SCALE).reshape(F, H, F)
    watt = np.ascontiguousarray(
        wa.transpose(2, 1, 0).reshape(F, H * F).astype(ml_dtypes.bfloat16))
    wbd = np.zeros((H * F, H * C), np.float32)
    wl = np.asarray(W_lin, dtype=np.float32)
    for h in range(H):
        wbd[h * F:(h + 1) * F, h * C:(h + 1) * C] = wl[:, h * C:(h + 1) * C]
    wbd = np.ascontiguousarray(
        wbd.reshape(2, 128, H * C).astype(ml_dtypes.bfloat16))
    biasb = np.ascontiguousarray(
        np.broadcast_to(np.asarray(bias, np.float32), (128, H * C)))
    xpadf = np.zeros((NPAD, F), np.float32)
    xpadf[:N] = np.asarray(x, np.float32)
    maps = []
    for c in range(NCORES):
        lo = c * NPC
        xwinT = np.ascontiguousarray(
            xpadf[lo:lo + NPC].reshape(NWIN, 128, F).transpose(0, 2, 1)
            .astype(ml_dtypes.bfloat16))
        maps.append({
            "xbf": xbf,
            "watt": watt,
            "wbd": wbd,
            "biasb": biasb,
            "itile": np.ascontiguousarray(itile[c]),
            "dstcol": np.ascontiguousarray(dstcol[c]),
            "wt": np.ascontiguousarray(wt[c]),
            "dstrow": np.ascontiguousarray(dstrow[c]),
            "xwinT": xwinT,
        })
    return maps


_last = None  # BassKernelResults of the most recent run (for test harness)


def kernel(x, edge_index, edge_weight, W_lin, W_att, bias):
    global _last
    S, OFS, TOT, itile, dstcol, wt, dstrow = _prep_edges(
        np.asarray(edge_index), np.asarray(edge_weight))
    nc = _build(S, OFS, TOT)
    in_maps = _make_in_maps(x, W_lin, W_att, bias, itile, dstcol, wt, dstrow)
    _last = run_bass_kernel_spmd(nc, in_maps, list(range(NCORES)))
    res = _last.results
    out = np.concatenate([res[c]["out"] for c in range(NCORES)], axis=0)
    return np.ascontiguousarray(out[:N])
